# revision 1
# baseline (speedup 1.0000x reference)
"""Trainium2 Bass kernel for an autoregressive LSTM decompressor.

Reference math (see problem):
  step 0:    gates = x @ W_ih.T + b            (h = c = 0)
  step t>=1: gates = h_{t-1} @ (W_ih+W_hh).T + b    (input == previous hidden)
  i,f,g,o = split(gates); c = sig(f)*c + sig(i)*tanh(g); h = sig(o)*tanh(c)
  out[b,t,:] = h_t @ W_out.T + b_out

Sharding: data-parallel, batch 256 -> 32 per core across 8 cores; weights
replicated. Per-core recurrence matmul per step: [32,1024] @ [1024,4096] done
as batch-stationary matmuls (lhsT = h.T chunk [128,32], moving = W.T [128,512]
fp32r full-rate). Bias is folded in as a K=1 matmul that also clears PSUM.
h is kept transposed (hT [128, 256] = 8 K-chunks of [128,32]) via DVE
stream-transpose + scatter DMA each step. h_t is streamed to DRAM each step;
the output projection runs once at the end as a big batched matmul.
"""

import os
import numpy as np

B, H, DOUT = 256, 1024, 128
NCORES = 8
BLOC = B // NCORES  # 32
G4 = 4 * H  # 4096

_CACHE = {}
_FLAGS = set()  # experiment flags: no_bias_mm, no_tail, no_seqproj
_REPS = 1  # timing experiments: repeat the steady-state loop


def _build_program(T):
    import concourse.mybir as mybir
    from concourse import bacc
    from concourse.tile import TileContext
    from concourse.masks import make_identity

    f32 = mybir.dt.float32
    f32r = mybir.dt.float32r
    if "bf16" in _FLAGS:
        f32r = mybir.dt.bfloat16  # matmul-operand dtype
    SIG = mybir.ActivationFunctionType.Sigmoid
    TANH = mybir.ActivationFunctionType.Tanh

    nc = bacc.Bacc("TRN2", target_bir_lowering=False, debug=False,
                   num_devices=NCORES)

    xT_d = nc.dram_tensor("xT", [H, BLOC], f32r, kind="ExternalInput").ap()
    wc_d = nc.dram_tensor("WcT", [H, G4], f32r, kind="ExternalInput").ap()
    wi_d = nc.dram_tensor("WiT", [H, G4], f32r, kind="ExternalInput").ap()
    biasr_d = nc.dram_tensor("biasr", [BLOC, G4], f32,
                             kind="ExternalInput").ap()
    wo_d = nc.dram_tensor("WoT", [H, DOUT], f32r, kind="ExternalInput").ap()
    bo_d = nc.dram_tensor("bo", [DOUT, 1], f32, kind="ExternalInput").ap()
    out_d = nc.dram_tensor("out", [BLOC, T, DOUT], f32,
                           kind="ExternalOutput").ap()
    # h_t history, transposed: [partition p, K-chunk k, t, b]
    seq_d = nc.dram_tensor("seq", [128, 8, T, BLOC], f32r, kind="Internal").ap()
    KDBG = bool(os.environ.get("KDBG"))
    if KDBG:
        gdbg_d = nc.dram_tensor("gdbg", [BLOC, G4], f32,
                                kind="ExternalOutput").ap()
        sdbg_d = nc.dram_tensor("sdbg", [128, 8, T, BLOC], f32,
                                kind="ExternalOutput").ap()

    with TileContext(nc) as tc:
        with (
            tc.tile_pool(name="const", bufs=1) as const_pool,
            tc.tile_pool(name="state", bufs=2) as state_pool,
            tc.tile_pool(name="cpool", bufs=1) as c_pool,
            tc.tile_pool(name="ew", bufs=2) as ew_pool,
        ):
            biasr_sb = const_pool.tile([BLOC, G4], f32, name="biasr_sb")
            nc.sync.dma_start(biasr_sb, biasr_d)
            bo_sb = const_pool.tile([DOUT, 1], f32, name="bo_sb")
            nc.sync.dma_start(bo_sb, bo_d)

            # initial hT = x.T, laid out as [128, 8*32] (chunk k at cols 32k)
            hT0 = state_pool.tile([128, 8 * BLOC], f32r, name="hT",
                                  tag="hT")
            for k in range(8):
                nc.sync.dma_start(hT0[:, 32 * k:32 * k + 32],
                                  xT_d[128 * k:128 * k + 128, :])

            c_sb = c_pool.tile([BLOC, H], f32, name="c_sb")
            nc.vector.memset(c_sb, 0.0)

            def lstm_tail(s, ps4, new_hT):
                """Elementwise + transpose for h-slice s. ps4 = (i, f, g, o)
                single-bank [BLOC, 512] gate PSUM tiles."""
                TANH_ = SIG if "all_sig" in _FLAGS else TANH
                ps_i, ps_f, ps_g, ps_o = ps4
                if_sb = ew_pool.tile([BLOC, 1024], f32, name="if_sb",
                                     tag="if_sb")
                nc.scalar.activation(if_sb[:, 0:512], ps_i, SIG)
                nc.scalar.activation(if_sb[:, 512:1024], ps_f, SIG)
                g_sb = ew_pool.tile([BLOC, 512], f32, name="g_sb", tag="g_sb")
                nc.scalar.activation(g_sb, ps_g, TANH_)
                o_sb = ew_pool.tile([BLOC, 512], f32, name="o_sb", tag="o_sb")
                nc.scalar.activation(o_sb, ps_o, SIG)

                csl = c_sb[:, 512 * s:512 * s + 512]
                i_sb = if_sb[:, 0:512]
                f_sb = if_sb[:, 512:1024]
                nc.vector.tensor_mul(i_sb, i_sb, g_sb)   # i*tanh(g)
                nc.vector.tensor_mul(f_sb, f_sb, csl)    # f*c
                nc.vector.tensor_add(csl, i_sb, f_sb)    # c_new
                nc.scalar.activation(g_sb, csl, TANH_)   # tanh(c_new)
                nc.vector.tensor_mul(o_sb, o_sb, g_sb)   # h slice

                # 32x32 block transpose, then scatter into hT layout
                scr = ew_pool.tile([BLOC, 512], f32, name="scr", tag="scr")
                nc.vector.transpose(scr, o_sb)
                scr_r = scr.bitcast(f32r).rearrange("q (kk j b) -> q kk j b",
                                                    j=4, b=BLOC)
                for j in range(4):
                    dst = new_hT[32 * j:32 * j + 32,
                                 128 * s:128 * s + 128]
                    nc.sync.dma_start(
                        dst.rearrange("q (k b) -> q k b", b=BLOC),
                        scr_r[:, :, j, :],
                    )

            with (
                tc.tile_pool(name="wc", bufs=1) as wc_pool,
                tc.tile_pool(name="gates_ps", bufs=8,
                             space="PSUM") as ps_pool,
            ):
                # resident combined weights, transposed+permuted
                wc_tiles = []
                for k in range(8):
                    w = wc_pool.tile([128, G4], f32r, name=f"wc{k}",
                                     tag=f"wc{k}")
                    wc_tiles.append(w)

                # ---- step 0: gates = x @ W_ih.T + b, streaming W_ih.T ----
                cur_hT = hT0
                new_hT = state_pool.tile([128, 8 * BLOC], f32r, name="hT",
                                         tag="hT")
                with tc.tile_pool(name="wi", bufs=2) as wi_pool:
                    ps_sl = [
                        tuple(ps_pool.tile([BLOC, 512], f32, name="gps",
                                           tag="gps") for _ in range(4))
                        for _ in range(2)]
                    for k in range(8):
                        # interleave: resident combined-weight load for later
                        nc.sync.dma_start(wc_tiles[k],
                                          wc_d[128 * k:128 * k + 128, :])
                        for hf in range(2):
                            wi_t = wi_pool.tile([128, 2048], f32r,
                                                name="wi_t", tag="wi_t")
                            nc.sync.dma_start(
                                wi_t, wi_d[128 * k:128 * k + 128,
                                           2048 * hf:2048 * hf + 2048])
                            for b4 in range(4):
                                nc.tensor.matmul(
                                    ps_sl[hf][b4],
                                    cur_hT[:, 32 * k:32 * k + 32],
                                    wi_t[:, 512 * b4:512 * b4 + 512],
                                    start=(k == 0), stop=(k == 7))
                    for s in range(2):
                        for g in range(4):
                            nc.vector.tensor_add(
                                ps_sl[s][g], ps_sl[s][g],
                                biasr_sb[:, 2048 * s + 512 * g:
                                         2048 * s + 512 * g + 512])
                    if KDBG:
                        for s in range(2):
                            for g in range(4):
                                gd = ew_pool.tile([BLOC, 512], f32,
                                                  name="gd", tag="gd")
                                nc.scalar.copy(gd, ps_sl[s][g])
                                nc.sync.dma_start(
                                    gdbg_d[:, 2048 * s + 512 * g:
                                           2048 * s + 512 * g + 512], gd)
                    for s in range(2):
                        lstm_tail(s, ps_sl[s], new_hT)
                    nc.sync.dma_start(
                        seq_d[:, :, 0, :],
                        new_hT.rearrange("p (k b) -> p k b", b=BLOC))
                    cur_hT = new_hT

                # ---- steps 1..T-1 with resident combined weights ----
                for t in list(range(1, T)) * _REPS:
                    new_hT = state_pool.tile([128, 8 * BLOC], f32r,
                                             name="hT", tag="hT")
                    ps_sl = [
                        tuple(ps_pool.tile([BLOC, 512], f32, name="gps",
                                           tag="gps") for _ in range(4))
                        for _ in range(2)]
                    # group A: K-chunks 0-3 (needs only h-slice-0 chunks,
                    # finished early by the previous step); then group B.
                    # Slice-1 bias comes in as a K=1 matmul (clears PSUM) so
                    # the late slice-1 chain skips a DVE hop; slice-0 bias
                    # stays on DVE (off the critical chain).
                    for s in range(2):
                        for g in range(4):
                            for k in range(4):
                                nc.tensor.matmul(
                                    ps_sl[s][g],
                                    cur_hT[:, 32 * k:32 * k + 32],
                                    wc_tiles[k][:, 2048 * s + 512 * g:
                                                2048 * s + 512 * g + 512],
                                    start=(k == 0), stop=False)
                    for s in range(2):
                        for g in range(4):
                            for k in range(4, 8):
                                nc.tensor.matmul(
                                    ps_sl[s][g],
                                    cur_hT[:, 32 * k:32 * k + 32],
                                    wc_tiles[k][:, 2048 * s + 512 * g:
                                                2048 * s + 512 * g + 512],
                                    start=False, stop=(k == 7))
                            nc.vector.tensor_add(
                                ps_sl[s][g], ps_sl[s][g],
                                biasr_sb[:, 2048 * s + 512 * g:
                                         2048 * s + 512 * g + 512])
                        if "no_tail" not in _FLAGS:
                            lstm_tail(s, ps_sl[s], new_hT)
                    if "no_tail" not in _FLAGS:
                        if "no_seqproj" not in _FLAGS:
                            nc.sync.dma_start(
                                seq_d[:, :, t, :],
                                new_hT.rearrange("p (k b) -> p k b", b=BLOC))
                        cur_hT = new_hT

            # ---- output projection: out.T = W_out @ h_all, then transpose
            with (
                tc.tile_pool(name="proj", bufs=3) as proj_pool,
                tc.tile_pool(name="projw", bufs=1) as projw_pool,
                tc.tile_pool(name="proj_ps", bufs=2, space="PSUM") as pps_pool,
                tc.tile_pool(name="tp_ps", bufs=2, space="PSUM") as tps_pool,
            ):
                ident = projw_pool.tile([128, 128], f32, name="ident")
                make_identity(nc, ident)
                wo_tiles = []
                for k in range(8):
                    wt = projw_pool.tile([128, DOUT], f32r, name=f"wo{k}",
                                         tag=f"wo{k}")
                    nc.sync.dma_start(wt, wo_d[128 * k:128 * k + 128, :])
                    wo_tiles.append(wt)

                nTB = T * BLOC  # total (t, b) rows
                n_chunks = max(1, nTB // 512)
                t_per_chunk = T // n_chunks
                cw = t_per_chunk * BLOC  # columns per chunk
                for n in range(n_chunks):
                    acc = pps_pool.tile([128, cw], f32, name="acc",
                                        tag="acc")
                    for k in range(8):
                        rhs = proj_pool.tile([128, cw], f32r, name="prhs",
                                             tag="prhs")
                        nc.sync.dma_start(
                            rhs.rearrange("p (t b) -> p t b", b=BLOC),
                            seq_d[:, k,
                                  t_per_chunk * n:t_per_chunk * (n + 1), :])
                        nc.tensor.matmul(acc, wo_tiles[k],
                                         rhs,
                                         start=(k == 0), stop=(k == 7))
                    osb = proj_pool.tile([128, cw], f32, name="osb",
                                         tag="osb")
                    nc.scalar.add(osb, acc, bo_sb)
                    for m in range(cw // 128):
                        tp = tps_pool.tile([128, 128], f32, name="tp",
                                           tag="tp")
                        nc.tensor.transpose(tp, osb[:, 128 * m:128 * m + 128],
                                            ident)
                        ob = proj_pool.tile([128, 128], f32, name="ob",
                                            tag="ob")
                        nc.vector.tensor_copy(ob, tp)
                        t0 = t_per_chunk * n + 4 * m
                        for tt in range(128 // BLOC):
                            nc.sync.dma_start(
                                out_d[:, t0 + tt, :],
                                ob[BLOC * tt:BLOC * tt + BLOC, :])
            if KDBG:
                nc.sync.dma_start(sdbg_d, seq_d.bitcast(f32))
    nc.finalize()
    return nc


def _gate_perm():
    # new gate-column order: [i0,f0,g0,o0,i1,f1,g1,o1], each 512 wide
    parts = []
    for s in range(2):
        for g in range(4):
            base = g * H + 512 * s
            parts.append(np.arange(base, base + 512))
    return np.concatenate(parts)


def _mm_np_dtype():
    if "bf16" in _FLAGS:
        import ml_dtypes
        return ml_dtypes.bfloat16
    return np.float32


def kernel(x, W_ih, W_hh, b_ih, b_hh, W_out, b_out, T):
    T = int(T)
    x = np.asarray(x, dtype=np.float32)
    W_ih = np.asarray(W_ih, dtype=np.float32)
    W_hh = np.asarray(W_hh, dtype=np.float32)
    b_ih = np.asarray(b_ih, dtype=np.float32)
    b_hh = np.asarray(b_hh, dtype=np.float32)
    W_out = np.asarray(W_out, dtype=np.float32)
    b_out = np.asarray(b_out, dtype=np.float32)

    from concourse.bass_utils import run_bass_kernel_spmd

    if T not in _CACHE:
        _CACHE[T] = _build_program(T)
    nc = _CACHE[T]

    mdt = _mm_np_dtype()
    perm = _gate_perm()
    WcT = np.ascontiguousarray((W_ih + W_hh)[perm].T.astype(mdt))
    WiT = np.ascontiguousarray(W_ih[perm].T.astype(mdt))
    biasr = np.ascontiguousarray(
        np.broadcast_to((b_ih + b_hh)[perm].reshape(1, G4), (BLOC, G4)))
    WoT = np.ascontiguousarray(W_out.T.astype(mdt))
    bo = np.ascontiguousarray(b_out.reshape(DOUT, 1))
    xT = np.ascontiguousarray(x.T.astype(mdt))

    in_maps = []
    for c in range(NCORES):
        in_maps.append({
            "xT": np.ascontiguousarray(xT[:, BLOC * c:BLOC * (c + 1)]),
            "WcT": WcT, "WiT": WiT, "biasr": biasr,
            "WoT": WoT, "bo": bo,
        })

    res = run_bass_kernel_spmd(nc, in_maps, core_ids=list(range(NCORES)))
    kernel.last_results = res.results
    out = np.concatenate([r["out"] for r in res.results], axis=0)
    return out



# revision 6
# speedup vs baseline: 1328.0425x; 1328.0425x over previous
"""Trainium2 Bass kernel for an autoregressive LSTM decompressor.

Math (see reference):
  step 0:    gates = x @ W_ih.T + b          (h = c = 0)
  step t>=1: gates = h_{t-1} @ (W_ih+W_hh).T + b
  i,f,g,o = split(gates); c = sig(f)*c + sig(i)*tanh(g); h = sig(o)*tanh(c)
  out[b,t,:] = h_t @ W_out.T + b_out

Data-parallel over 8 cores (batch 32 each), weights replicated, bf16
matmul operands (rel err ~3e-3, tolerance 2e-2).

Per-core step: gates for hidden-slice s (512 cols) live in one PSUM tile
[128, 512]: partition group 32g holds gate q of order [i, f, o, g]
(host-permuted weight columns). Each K-chunk k is a "wave" of 4
column-tiled matmuls (tile_position=(0,32g)) running concurrently on the
PE array (~193ns per wave vs 4x216 serial). Bias enters as a K=4 matmul
with an indicator stationary that also clears PSUM. Tail: 2 ACT ops
(sigmoid on partitions 0:96, tanh g -> base 0), DVE muls in bf16 at
matched base partitions (c kept at partitions 32:64), tanh(c) -> base
64, h -> base 0. h is transposed back to hT layout via 4 cheap PE
transposes into one PSUM tile + a single merged [128,128] DVE copy --
no DMA on the recurrence chain. seq history goes to DRAM per step; the
output projection at the end computes out.T = W_out @ h and stores it
untransposed (host transposes).

Slice-1's transposes are deferred into the next step's PE stream
(after a few waves) so the PE never stalls on the elementwise chain.
"""

import numpy as np

B, H, DOUT = 256, 1024, 128
NCORES = 8
BLOC = B // NCORES  # 32
G4 = 4 * H

_CACHE = {}
_FLAGS = set()
_REPS = 1

# torch gate order rows: i [0,1024) f [1,2) g [2,3) o [3,4)*1024
# partition-group order in PSUM: [i, f, o, g]
GATE_BASE = [0, 1024, 3072, 2048]


def _gate_perm():
    parts = []
    for s in range(2):
        for q in range(4):
            base = GATE_BASE[q] + 512 * s
            parts.append(np.arange(base, base + 512))
    return np.concatenate(parts)


def _build_program(T):
    import concourse.mybir as mybir
    from concourse import bacc
    from concourse.tile import TileContext
    from concourse.masks import make_identity

    f32 = mybir.dt.float32
    bf16 = mybir.dt.bfloat16
    SIG = mybir.ActivationFunctionType.Sigmoid
    TANH = mybir.ActivationFunctionType.Tanh
    ONE_MM = "sim1mm" in _FLAGS  # sim-tuning: 1 matmul per wave

    nc = bacc.Bacc("TRN2", target_bir_lowering=False, debug=False,
                   num_devices=NCORES)

    xT_d = nc.dram_tensor("xT", [H, BLOC], bf16, kind="ExternalInput").ap()
    wc_d = nc.dram_tensor("WcT", [H, G4], bf16, kind="ExternalInput").ap()
    wi_d = nc.dram_tensor("WiT", [H, G4], bf16, kind="ExternalInput").ap()
    bq0_d = nc.dram_tensor("biasq0", [4, 512], bf16, kind="ExternalInput").ap()
    bq1_d = nc.dram_tensor("biasq1", [4, 512], bf16, kind="ExternalInput").ap()
    bind_d = nc.dram_tensor("bind", [4, 128], bf16, kind="ExternalInput").ap()
    wo_d = nc.dram_tensor("WoT", [H, DOUT], bf16, kind="ExternalInput").ap()
    bo_d = nc.dram_tensor("bo", [DOUT, 1], f32, kind="ExternalInput").ap()
    outT_d = nc.dram_tensor("outT", [DOUT, T, BLOC], f32,
                            kind="ExternalOutput").ap()
    seq_d = nc.dram_tensor("seq", [128, 8, T, BLOC], bf16,
                           kind="Internal").ap()

    with TileContext(nc) as tc:
        with (
            tc.tile_pool(name="const", bufs=1) as const_pool,
            tc.tile_pool(name="state", bufs=2) as state_pool,
            tc.tile_pool(name="cpool", bufs=1) as c_pool,
            tc.tile_pool(name="act", bufs=2) as act_pool,
            tc.tile_pool(name="ew", bufs=2) as ew_pool,
        ):
            bq_sb = []
            for s, d in ((0, bq0_d), (1, bq1_d)):
                t_ = const_pool.tile([4, 512], bf16, name=f"bq{s}")
                nc.sync.dma_start(t_, d)
                bq_sb.append(t_)
            bind_sb = const_pool.tile([4, 128], bf16, name="bind_sb")
            nc.sync.dma_start(bind_sb, bind_d)
            bo_sb = const_pool.tile([DOUT, 1], f32, name="bo_sb")
            nc.sync.dma_start(bo_sb, bo_d)
            ident32 = const_pool.tile([32, 32], bf16, name="ident32")
            make_identity(nc, ident32)

            c_sb = c_pool.tile([64, H], bf16, name="c_sb")  # rows 32:64 used
            nc.vector.memset(c_sb[32:64, :], 0.0)

            def new_hT():
                return state_pool.tile([128, 8 * BLOC], bf16, name="hT",
                                       tag="hT")

            hT0 = new_hT()
            for k in range(8):
                nc.sync.dma_start(hT0[:, 32 * k:32 * k + 32],
                                  xT_d[128 * k:128 * k + 128, :])

            def bias_mm(ps, s):
                nc.tensor.matmul(ps, bind_sb, bq_sb[s], start=True,
                                 stop=False, skip_group_check=True)

            def wave(ps, s, k, hT_prev, wsrc, last):
                ngr = 1 if ONE_MM else 4
                for g in range(ngr):
                    nc.tensor.matmul(
                        ps[32 * g:32 * g + 32, :],
                        hT_prev[:, 32 * k:32 * k + 32],
                        wsrc[:, 2048 * s + 512 * g:2048 * s + 512 * g + 512],
                        start=False, stop=last,
                        tile_position=(0, 32 * g), skip_group_check=True)

            def tail(s, ps):
                """gates PSUM -> h_sb [32,512] bf16 at base 0."""
                act = act_pool.tile([128, 512], bf16, name=f"act{s}",
                                    tag=f"act{s}")
                nc.scalar.activation(act[0:96, :], ps[0:96, :], SIG)
                g_sb = ew_pool.tile([32, 512], bf16, name=f"g{s}",
                                    tag=f"g{s}")
                nc.scalar.activation(g_sb, ps[96:128, :], TANH)
                ig = ew_pool.tile([32, 512], bf16, name=f"ig{s}",
                                  tag=f"ig{s}")
                nc.vector.tensor_mul(ig, act[0:32, :], g_sb)
                csl = c_sb[32:64, 512 * s:512 * s + 512]
                fc = ew_pool.tile([32, 512], bf16, name=f"fc{s}",
                                  tag=f"fc{s}")
                nc.vector.tensor_mul(fc, act[32:64, :], csl)
                nc.vector.tensor_add(csl, ig, fc)
                th = ew_pool.tile([96, 512], bf16, name=f"th{s}",
                                  tag=f"th{s}")  # rows 64:96 used
                nc.scalar.activation(th[64:96, :], csl, TANH)
                h_sb = ew_pool.tile([32, 512], bf16, name=f"h{s}",
                                    tag=f"h{s}")
                nc.vector.tensor_mul(h_sb, act[64:96, :], th[64:96, :])
                return h_sb

            def transposes(s, h_sb, hT_dst):
                """4 PE transposes + 1 merged copy: h slice -> hT chunks."""
                tp = tp_pool.tile([128, 128], bf16, name=f"tp{s}",
                                  tag=f"tp{s}")
                for j in range(4):
                    nc.tensor.transpose(tp[:, 32 * j:32 * j + 32],
                                        h_sb[:, 128 * j:128 * j + 128],
                                        ident32)
                nc.vector.tensor_copy(hT_dst[:, 128 * s:128 * s + 128], tp)

            def seq_store(t, hT_t):
                nc.sync.dma_start(
                    seq_d[:, :, t, :],
                    hT_t.rearrange("p (k b) -> p k b", b=BLOC))

            with (
                tc.tile_pool(name="wc", bufs=1) as wc_pool,
                tc.tile_pool(name="gates_ps", bufs=2,
                             space="PSUM") as ps_pool,
                tc.tile_pool(name="tp_ps", bufs=2, space="PSUM") as tp_pool,
            ):
                wc_tiles = [wc_pool.tile([128, G4], bf16, name=f"wc{k}",
                                         tag=f"wc{k}") for k in range(8)]

                # ---- step 0: stream W_ih; also load resident Wc ----
                cur_hT = hT0
                pending = None  # (h_sb_s1, hT_t, t) awaiting transposes
                with tc.tile_pool(name="wi", bufs=3) as wi_pool:
                    ps = [ps_pool.tile([128, 512], f32, name=f"gps{s}",
                                       tag=f"gps{s}") for s in range(2)]
                    bias_mm(ps[0], 0)
                    bias_mm(ps[1], 1)
                    for k in range(8):
                        wi_t = wi_pool.tile([128, G4], bf16, name="wi_t",
                                            tag="wi_t")
                        nc.sync.dma_start(wi_t,
                                          wi_d[128 * k:128 * k + 128, :])
                        wave(ps[0], 0, k, cur_hT, wi_t, last=(k == 7))
                        wave(ps[1], 1, k, cur_hT, wi_t, last=(k == 7))
                        nc.sync.dma_start(wc_tiles[k],
                                          wc_d[128 * k:128 * k + 128, :])
                    hT_t = new_hT()
                    h0 = tail(0, ps[0])
                    transposes(0, h0, hT_t)
                    h1 = tail(1, ps[1])
                    pending = (h1, hT_t, 0)
                    cur_hT = hT_t

                # ---- steps 1..T-1 ----
                for t in list(range(1, T)) * _REPS:
                    ps = [ps_pool.tile([128, 512], f32, name=f"gps{s}",
                                       tag=f"gps{s}") for s in range(2)]
                    hT_t = new_hT()
                    bias_mm(ps[0], 0)
                    bias_mm(ps[1], 1)
                    # s0 waves k=0..2
                    for k in range(3):
                        wave(ps[0], 0, k, cur_hT, wc_tiles[k], last=False)
                    # deferred: previous step's s1 transposes + seq store
                    if pending is not None:
                        ph, phT, pt = pending
                        transposes(1, ph, phT)
                        seq_store(pt, phT)
                    for k in range(3, 8):
                        wave(ps[0], 0, k, cur_hT, wc_tiles[k],
                             last=(k == 7))
                    for k in range(8):
                        wave(ps[1], 1, k, cur_hT, wc_tiles[k],
                             last=(k == 7))
                    h0 = tail(0, ps[0])
                    transposes(0, h0, hT_t)
                    h1 = tail(1, ps[1])
                    pending = (h1, hT_t, t)
                    cur_hT = hT_t

                # flush pending
                ph, phT, pt = pending
                transposes(1, ph, phT)
                seq_store(pt, phT)

            # ---- output projection: outT = W_out @ h_all ----
            with (
                tc.tile_pool(name="proj", bufs=3) as proj_pool,
                tc.tile_pool(name="projw", bufs=1) as projw_pool,
                tc.tile_pool(name="proj_ps", bufs=2, space="PSUM") as pps,
            ):
                wo_tiles = []
                for k in range(8):
                    wt = projw_pool.tile([128, DOUT], bf16, name=f"wo{k}",
                                         tag=f"wo{k}")
                    nc.sync.dma_start(wt, wo_d[128 * k:128 * k + 128, :])
                    wo_tiles.append(wt)

                n_chunks = max(1, (T * BLOC) // 512)
                t_per = T // n_chunks
                cw = t_per * BLOC
                for n in range(n_chunks):
                    acc = pps.tile([128, cw], f32, name="acc", tag="acc")
                    for k in range(8):
                        rhs = proj_pool.tile([128, cw], bf16, name="prhs",
                                             tag="prhs")
                        nc.sync.dma_start(
                            rhs.rearrange("p (t b) -> p t b", b=BLOC),
                            seq_d[:, k, t_per * n:t_per * (n + 1), :])
                        nc.tensor.matmul(acc, wo_tiles[k], rhs,
                                         start=(k == 0), stop=(k == 7))
                    osb = proj_pool.tile([128, cw], f32, name="osb",
                                         tag="osb")
                    nc.scalar.add(osb, acc, bo_sb)
                    nc.sync.dma_start(
                        outT_d[:, t_per * n:t_per * (n + 1), :],
                        osb.rearrange("p (t b) -> p t b", b=BLOC))
    nc.finalize()
    return nc


def _prep_inmaps(x, W_ih, W_hh, b_ih, b_hh, W_out, b_out):
    import ml_dtypes
    bf = ml_dtypes.bfloat16
    x = np.asarray(x, dtype=np.float32)
    W_ih = np.asarray(W_ih, dtype=np.float32)
    W_hh = np.asarray(W_hh, dtype=np.float32)
    b = (np.asarray(b_ih, dtype=np.float32)
         + np.asarray(b_hh, dtype=np.float32))
    W_out = np.asarray(W_out, dtype=np.float32)
    b_out = np.asarray(b_out, dtype=np.float32)

    perm = _gate_perm()
    WcT = np.ascontiguousarray((W_ih + W_hh)[perm].T.astype(bf))
    WiT = np.ascontiguousarray(W_ih[perm].T.astype(bf))
    biasq = b[perm].reshape(2, 4, 512).astype(bf)
    bind = np.zeros((4, 128), np.float32)
    for q in range(4):
        bind[q, 32 * q:32 * q + 32] = 1.0
    bind = bind.astype(bf)
    WoT = np.ascontiguousarray(W_out.T.astype(bf))
    bo = np.ascontiguousarray(b_out.reshape(DOUT, 1))
    xT = np.ascontiguousarray(x.T.astype(bf))

    in_maps = []
    for c in range(NCORES):
        in_maps.append({
            "xT": np.ascontiguousarray(xT[:, BLOC * c:BLOC * (c + 1)]),
            "WcT": WcT, "WiT": WiT,
            "biasq0": np.ascontiguousarray(biasq[0]),
            "biasq1": np.ascontiguousarray(biasq[1]),
            "bind": bind, "WoT": WoT, "bo": bo,
        })
    return in_maps


def kernel(x, W_ih, W_hh, b_ih, b_hh, W_out, b_out, T):
    T = int(T)
    from concourse.bass_utils import run_bass_kernel_spmd

    if T not in _CACHE:
        _CACHE[T] = _build_program(T)
    nc = _CACHE[T]

    in_maps = _prep_inmaps(x, W_ih, W_hh, b_ih, b_hh, W_out, b_out)
    res = run_bass_kernel_spmd(nc, in_maps, core_ids=list(range(NCORES)))
    kernel.last_results = res.results
    out = np.concatenate(
        [np.ascontiguousarray(r["outT"].transpose(2, 1, 0))
         for r in res.results], axis=0)
    return out
